# revision 12
# baseline (speedup 1.0000x reference)
"""Hex-masked sparse conv (ConvHex) as a Bass/Tile kernel on 8 TRN2 NeuronCores.

Strategy
--------
Data-parallel over batch: 16 images -> 2 per core.

The conv has 19 hex taps in a 9x5 window, C_in=64, C_out=128. The output
mask is hex-shaped and parity-sparse: row h only has active outputs at
columns w with (h+w) even, stride-2 contiguous. We therefore only compute
the stride-2 active lattice (~half the naive output columns).

On device, per image, x is stored channel-major in SBUF as a [128, HW]
bf16 tile whose partitions 0:64 hold x^T (channels x flattened positions)
and partitions 64:128 hold the same data shifted by S = 4*W positions.
This lets us pack PAIRS of taps (dh, dw) and (dh+4, dw) into single
K=128 matmuls (8 pairs + 3 singleton K=64 taps = 11 matmul streams).

v2 changes vs baseline:
 - x is loaded from HBM ONCE (3.55MB/image) in row-chunks; the shifted
   upper half is built by SBUF->SBUF DMA copies per chunk. Matmuls start
   as soon as the first chunks land (subtile deps), hiding the input load.
 - per-GROUP pipelining (8 PSUM banks deep) instead of 7-group blocks.
 - epilogue in bf16 (2x DVE) and output stored group-packed
   [NPER, COUT, TOTF] bf16: one DMA per group = 128 descriptors of
   nf*2 contiguous bytes (vs 54k tiny strided writes). Host unpacks.
 - output DMAs issued via gpsimd SWDGE to keep the SP sequencer free.

Each matmul computes a group of up to 7 same-parity output rows at once
(free AP [rows, cols] with steps [2*W, 2]), accumulating 11 streams into
one PSUM bank. Epilogue: device computes elu(z)+1 = min(exp(z),1)+relu(z)
(z = conv + offset) with one ScalarE exp, one ScalarE relu and one
VectorE STT; the host subtracts the 1 during reassembly.
"""

import numpy as np
import ml_dtypes

# ---------------------------------------------------------------- constants
R = 2
CIN, COUT = 64, 128
H, W = 209, 133
HW = H * W                      # 27797
OH, OW = H - 4 * R, W - 2 * R   # 201, 129
NBATCH, NCORES = 16, 8
NPER = NBATCH // NCORES         # 2 images per core
DH_SHIFT = 4                    # pair taps (dh, dw) with (dh+4, dw)
SHIFT = DH_SHIFT * W            # 532 flattened positions
NROWS = 7                       # output rows per matmul group
SLOTS = 65                      # max stride-2 columns per output row
PAD = 2 * W * NROWS             # sbuf free-dim padding so row-slab APs stay in bounds
RCH = 28                        # input rows per DMA chunk

BF16 = ml_dtypes.bfloat16


def _hex_indices(radius):
    moves = np.array([[1, 1], [2, 0], [1, -1], [-1, -1], [-2, 0], [-1, 1]])
    out = [[2 * radius, radius]]
    for il in range(1, radius + 1):
        s = np.array([[2 * radius - 2 * il, radius]])
        cur = moves.repeat(il, axis=0).cumsum(axis=0)
        out.extend((s + cur).tolist())
    return np.array(out, dtype=np.int32)


def _make_out_mask():
    mr = (OW - 1) // 2
    f = np.zeros((mr * 4 + 1, mr * 2 + 1), dtype=np.float32)
    for ind in _hex_indices(mr):
        f[tuple(ind)] = 1.0
    i_cut = (mr * 4 + 1 - OH) // 2
    return f[i_cut:-i_cut, :]    # [OH, OW]


_TAPS = _hex_indices(R)          # [19, 2] (dh, dw), reference tap order j
_NTAPS = len(_TAPS)
_MASK = _make_out_mask()         # [201, 129] float32


def _make_streams():
    """Pair taps (dh,dw) with (dh+4,dw). Returns (pairs, singles) as tap indices."""
    idx = {tuple(t): j for j, t in enumerate(_TAPS.tolist())}
    used = set()
    pairs, singles = [], []
    # walk chains along +(4,0) so each tap pairs at most once
    for t in sorted(idx):
        if t in used or (t[0] - DH_SHIFT, t[1]) in idx:
            continue
        chain = [t]
        cur = t
        while (cur[0] + DH_SHIFT, cur[1]) in idx:
            cur = (cur[0] + DH_SHIFT, cur[1])
            chain.append(cur)
        for k in range(0, len(chain) - 1, 2):
            pairs.append((idx[chain[k]], idx[chain[k + 1]]))
            used.update(chain[k:k + 2])
        if len(chain) % 2:
            singles.append(idx[chain[-1]])
            used.add(chain[-1])
    assert sum(1 for _ in pairs) * 2 + len(singles) == _NTAPS
    return pairs, singles


_PAIRS, _SINGLES = _make_streams()   # 8 pairs + 3 singles
# pack two of the three leftover singles, (4,0)+(4,4), into a 9th pair fed
# from a second SBUF tile whose upper half is x shifted by 4 columns
_TIDX = {tuple(t): j for j, t in enumerate(_TAPS.tolist())}
_BPAIR = (_TIDX[(4, 0)], _TIDX[(4, 4)])
_SINGLES = [_TIDX[(8, 2)]]
BSHIFT = 4                           # flattened-position shift for the B pair
_NSTREAMS = len(_PAIRS) + len(_SINGLES) + 1   # 8 A-pairs + 1 single + 1 B-pair


def _make_groups():
    """Groups of <=NROWS same-parity output rows sharing one PSUM bank.

    Returns list of (h0, nrows, k0, ncols) where the group covers output rows
    h0, h0+2, ..., h0+2*(nrows-1) and slots k0..k0+ncols-1 of the compact
    65-slot row layout (slot k of row h <-> w = (h%2) + 2*k).
    """
    spans = []
    for h in range(OH):
        w_act = np.nonzero(_MASK[h])[0]
        spans.append((int(w_act[0]), int(w_act[-1])))
    groups = []
    for p in (0, 1):
        rows = list(range(p, OH, 2))
        for i in range(0, len(rows), NROWS):
            chunk = rows[i:i + NROWS]
            w_lo = min(spans[h][0] for h in chunk)
            w_hi = max(spans[h][1] for h in chunk)
            k0 = (w_lo - p) // 2
            ncols = (w_hi - w_lo) // 2 + 1
            groups.append((chunk[0], len(chunk), k0, ncols))
    return groups


_GROUPS = _make_groups()
# process groups in ascending h0 so matmuls chase the chunked input load
_ORDER = sorted(range(len(_GROUPS)), key=lambda i: _GROUPS[i][0])
_OFFS = {}
_off = 0
for _gi in _ORDER:
    _OFFS[_gi] = _off
    _off += _GROUPS[_gi][1] * _GROUPS[_gi][3]
TOTF = _off                      # packed output positions per image
MAXNF = max(nr * nc for (_h, nr, _k, nc) in _GROUPS)

# input chunk boundaries (flattened positions), row-aligned; short first
# chunk so the first groups' matmuls can start sooner
_ROW_BOUNDS = [0, 14] + list(range(42, H, RCH)) + [H]
_CHUNKS = [(_ROW_BOUNDS[i] * W, _ROW_BOUNDS[i + 1] * W)
           for i in range(len(_ROW_BOUNDS) - 1)]


# ---------------------------------------------------------------- device program
_PROGRAM = None


def _build_program():
    import concourse.mybir as mybir
    from concourse import bacc
    from concourse.tile import TileContext

    f32 = mybir.dt.float32
    bf16 = mybir.dt.bfloat16
    Alu = mybir.AluOpType
    Act = mybir.ActivationFunctionType

    # Bacc (not plain Bass): its compile() legalizes sync waits for the
    # TRN2 1-wait-per-instruction limit via generate_event_semaphores
    nc = bacc.Bacc("TRN2", target_bir_lowering=False, debug=False)
    xt_in = nc.declare_dram_parameter("xt", [NPER, CIN, HW], bf16, isOutput=False)
    wp_in = nc.declare_dram_parameter("wp", [len(_PAIRS), 128, COUT], bf16, isOutput=False)
    wb_in = nc.declare_dram_parameter("wb", [128, COUT], bf16, isOutput=False)
    ws_in = nc.declare_dram_parameter("ws", [len(_SINGLES), CIN, COUT], bf16, isOutput=False)
    bias_in = nc.declare_dram_parameter("bias", [COUT, 1], f32, isOutput=False)
    out_p = nc.declare_dram_parameter("out", [NPER, COUT, TOTF], bf16, isOutput=True)

    with TileContext(nc) as tc:
        with (
            tc.tile_pool(name="const", bufs=1) as cpool,
            tc.tile_pool(name="x", bufs=2) as xpool,
            tc.tile_pool(name="xb", bufs=1) as xbpool,
            tc.tile_pool(name="ps", bufs=8, space="PSUM") as pspool,
            tc.tile_pool(name="ep", bufs=4) as epool,
            tc.tile_pool(name="rp", bufs=4) as rpool,
            tc.tile_pool(name="sp", bufs=4) as spool,
        ):
            xts = []

            def load_image(n):
                """HBM loads only, on the SP (sync) HWDGE queue."""
                xt_t = xpool.tile([128, HW + PAD], bf16, name="xt", tag="xt")
                xts.append(xt_t)
                for (a, b) in _CHUNKS:
                    nc.sync.dma_start(xt_t[0:CIN, a:b], xt_in[n, :, a:b])
                return xt_t

            def copy_image(n):
                """SBUF->SBUF shifted-copy chunks, on the ACT HWDGE queue so
                their load-completion waits never block the SP queue from
                issuing further loads."""
                xt_t = xts[n]
                for (a, b) in _CHUNKS:
                    a2 = max(a - SHIFT, 0)
                    b2 = b - SHIFT
                    nc.scalar.dma_start(xt_t[CIN:128, a2:b2],
                                        xt_t[0:CIN, a2 + SHIFT:b2 + SHIFT])

            def copy_image_b(n):
                """Build xb for image n from its x tile: lower = x, upper =
                x shifted by BSHIFT columns. gpsimd SWDGE queue; single
                buffered — subtile WAR deps gate chunk-by-chunk on the
                previous image's B matmuls having read that range."""
                xt_t = xts[n]
                for (a, b) in _CHUNKS:
                    nc.gpsimd.dma_start(xb_t[0:CIN, a:b], xt_t[0:CIN, a:b])
                    a4 = max(a - BSHIFT, 0)
                    b4 = b - BSHIFT
                    nc.gpsimd.dma_start(xb_t[CIN:128, a4:b4],
                                        xt_t[0:CIN, a4 + BSHIFT:b4 + BSHIFT])

            # bias and weights first (small, fast) so they never queue behind
            # the bulk image loads in the in-order SP HWDGE queue
            bias_t = cpool.tile([COUT, 1], f32)
            nc.sync.dma_start(bias_t[:], bias_in[:])
            wp_t = cpool.tile([128, len(_PAIRS) * COUT], bf16)
            for s in range(len(_PAIRS)):
                nc.sync.dma_start(wp_t[:, s * COUT:(s + 1) * COUT], wp_in[s])
            wb_t = cpool.tile([128, COUT], bf16)
            nc.sync.dma_start(wb_t[:], wb_in[:])
            ws_t = cpool.tile([CIN, len(_SINGLES) * COUT], bf16)
            for s in range(len(_SINGLES)):
                nc.sync.dma_start(ws_t[:, s * COUT:(s + 1) * COUT], ws_in[s])
            load_image(0)
            load_image(1)

            xb_t = xbpool.tile([128, HW + PAD], bf16, name="xb", tag="xb")
            copy_image(0)
            copy_image_b(0)
            # warmup activations: preload the ACT function tables and absorb
            # the bias-DMA wait so no steady-state ACT needs >2 sync waits
            warm_t = cpool.tile([1, 1], f32)
            nc.scalar.activation(warm_t[0:1, 0:1], bias_t[0:1, 0:1], Act.Exp)
            nc.scalar.activation(warm_t[0:1, 0:1], bias_t[0:1, 0:1], Act.Relu)

            def do_image(n, mid_hook=None):
                xt_t = xts[n]
                for gn, gi in enumerate(_ORDER):
                    if gn == 24 and mid_hook is not None:
                        mid_hook()
                    h0, nrows, k0, ncols = _GROUPS[gi]
                    nf = nrows * ncols
                    ps = pspool.tile([128, 512], f32, name="ps", tag="ps")
                    pv = ps[:, 0:nf].rearrange("p (h w) -> p h w", h=nrows)

                    def rhs_ap(src_t, dh, dw, kpart):
                        o0 = (h0 + dh) * W + dw + (k0 * 2 + (h0 % 2))
                        sl = src_t[0:kpart, o0:o0 + 2 * W * nrows]
                        return sl.rearrange("p (h q) -> p h q", h=nrows)[:, :, 0:2 * ncols:2]

                    si = 0
                    for (i1, _i2) in _PAIRS:
                        dh, dw = map(int, _TAPS[i1])
                        nc.tensor.matmul(
                            pv, wp_t[:, si * COUT:(si + 1) * COUT],
                            rhs_ap(xt_t, dh, dw, 128),
                            start=(si == 0), stop=False)
                        si += 1
                    for ss, j in enumerate(_SINGLES):
                        dh, dw = map(int, _TAPS[j])
                        nc.tensor.matmul(
                            pv, ws_t[:, ss * COUT:(ss + 1) * COUT],
                            rhs_ap(xt_t, dh, dw, CIN),
                            start=False, stop=False)
                        si += 1
                    # B pair last: maximum slack for the single-buffered xb
                    dh, dw = map(int, _TAPS[_BPAIR[0]])
                    nc.tensor.matmul(
                        pv, wb_t[:, 0:COUT], rhs_ap(xb_t, dh, dw, 128),
                        start=False, stop=True)

                    e_t = epool.tile([128, MAXNF], bf16, name="et", tag="et")
                    a_t = rpool.tile([128, MAXNF], bf16, name="at", tag="at")
                    # spare columns (index MAXNF..) for the disjoint
                    # slot-acquire pre-write
                    s_t = spool.tile([128, MAXNF + 16], bf16, name="st", tag="st")
                    # device computes elu(z)+1 = min(exp(z),1) + relu(z),
                    # z = conv + bias; the host subtracts the 1 during
                    # reassembly. ACT is the only PSUM reader and DVE the
                    # only pre-DMA writer, keeping every instruction
                    # within its ISA sync-wait slot budget.
                    nc.scalar.activation(e_t[:, 0:nf], ps[:, 0:nf], Act.Exp,
                                         bias=bias_t[:, 0:1])
                    nc.scalar.activation(a_t[:, 0:nf], ps[:, 0:nf], Act.Relu,
                                         bias=bias_t[:, 0:1])
                    # disjoint 1-element pre-write takes the s_t
                    # slot-recycle waits, so the STT instruction (1
                    # sync-wait slot in its ISA struct) carries only the
                    # merged ACT wait
                    nc.vector.memset(s_t[0:1, MAXNF:], 0.0)
                    nc.vector.scalar_tensor_tensor(
                        s_t[:, 0:nf], e_t[:, 0:nf], 1.0, a_t[:, 0:nf],
                        op0=Alu.min, op1=Alu.add)
                    off = _OFFS[gi]
                    nc.gpsimd.dma_start(out_p[n, :, off:off + nf], s_t[:, 0:nf])

            # image-1 shift-copies are emitted late in image 0's group loop:
            # by then their source loads have finished, so their waits don't
            # stall the ACT queue ahead of image 0's epilogue activations
            do_image(0, mid_hook=lambda: copy_image(1))
            # xb rebuild for image 1 goes after image 0's groups: its WAR
            # waits (image 0's B matmuls) would otherwise block image 0's
            # output DMAs in the in-order gpsimd queue
            copy_image_b(1)
            do_image(1)
    nc.compile()
    return nc


def _get_program():
    global _PROGRAM
    if _PROGRAM is None:
        _PROGRAM = _build_program()
    return _PROGRAM


# ---------------------------------------------------------------- host wrapper
def _prep_inputs(x, sparse_weights, offset):
    xt = np.ascontiguousarray(x.transpose(0, 3, 1, 2)).reshape(NBATCH, CIN, HW)
    xt = xt.astype(BF16)
    sw3 = np.asarray(sparse_weights, np.float32).reshape(CIN, COUT, _NTAPS)
    wp = np.empty((len(_PAIRS), 128, COUT), np.float32)
    for s, (i1, i2) in enumerate(_PAIRS):
        wp[s, 0:CIN] = sw3[:, :, i1]
        wp[s, CIN:128] = sw3[:, :, i2]
    wb = np.empty((128, COUT), np.float32)
    wb[0:CIN] = sw3[:, :, _BPAIR[0]]
    wb[CIN:128] = sw3[:, :, _BPAIR[1]]
    ws = np.empty((len(_SINGLES), CIN, COUT), np.float32)
    for s, j in enumerate(_SINGLES):
        ws[s] = sw3[:, :, j]
    bias = np.asarray(offset, np.float32).reshape(COUT, 1)
    return xt, wp.astype(BF16), wb.astype(BF16), ws.astype(BF16), bias


def kernel(x, sparse_weights, offset):
    from concourse.bass_utils import run_bass_kernel_spmd

    xt, wp, wb, ws, bias = _prep_inputs(x, sparse_weights, offset)
    nc = _get_program()
    in_maps = [
        {"xt": xt[c * NPER:(c + 1) * NPER], "wp": wp, "wb": wb, "ws": ws,
         "bias": bias}
        for c in range(NCORES)
    ]
    res = run_bass_kernel_spmd(nc, in_maps, list(range(NCORES)))
    arr = np.concatenate([np.asarray(res.results[c]["out"])
                          for c in range(NCORES)], axis=0)   # [16, 128, TOTF] bf16
    arr = arr.astype(np.float32)
    full = np.zeros((NBATCH, OH, OW, COUT), np.float32)
    for gi in _ORDER:
        h0, nrows, k0, ncols = _GROUPS[gi]
        off = _OFFS[gi]
        p = h0 % 2
        blk = arr[:, :, off:off + nrows * ncols].reshape(NBATCH, COUT, nrows, ncols)
        # device returns elu(z) + 1
        full[:, h0:h0 + 2 * nrows:2, p + 2 * k0:p + 2 * (k0 + ncols):2, :] = (
            blk.transpose(0, 2, 3, 1) - 1.0)
    full[:, _MASK == 0] = 0.0
    return full
